# revision 13
# baseline (speedup 1.0000x reference)
"""MultiHeadEMABlock Trainium2 kernel (8-core SPMD, bass/Tile) — v3.

Math (reference):
  h = LayerNorm_c(x[b,c,n] over c) * gamma + beta          (per (b,n))
  xe[b,n,h,d] = h[b,n,d] * expansion[h,d]
  y = causal damped EMA along n: y[t] = a_h*sum_{s<=t} q_h^{t-s} xe[s]
  out[b,d,n] = sum_h y[b,n,h,d]*reduction[h,d] + x

Identities used:
  - Per-(h,d) scales commute with the EMA: out = x + sum_h rho_h[d]*S_h[d,n].
  - RANK-5 BASIS: the 8 exponential kernels a_h q_h^l (l in [0,160)) lie in a
    rank-5 subspace (SVD, max per-head rel err 1.9e-4). With basis phi_b and
    per-channel coefficients c_b[d] = sum_h rho_h[d] beta[h,b], the head sum
    collapses to 5 "basis heads":
      out_ema[d,t] = sum_b c_b[d] * (phi_b (*) z)[d,t]
  - q_max^128 ~ 1e-31, so each 128-chunk needs only its own + the previous
    chunk as history: cross-chunk carry state is replaced by a second
    triangular matmul (PHI2) against the PREVIOUS chunk's transposed inputs.
    No serial carry chain at all.
  - rstd is position-wise so it commutes with the c->n transpose: applied as
    a per-partition scale while evacuating the transposed PSUM.
  - beta(LN) contributes a data-independent term added on host (exact).

Sharding: 8 cores = 4 batches x 2 sequence halves, W=128 left halo.

Device algorithm (per core, c-major [channel x n] base layout):
  1. x loaded via SWDGE cast-DMA (f32->bf16). Mean via ones-matmul
     (replicated); zc = xb - mean on GpSimd. Position-column stats via tiny
     N=1 matmuls; r_col = exp(-.5 ln(var+eps)) on ACT over [128, nk] tiles.
  2. Per chunk: one LDW per dtile serves two scale+transpose matmuls (basis
     0-3 diag rhs N=512, basis-4 plane N=128); PSUM evacuated with the
     per-partition r_col scale fused (DVE tensor_scalar / ACT act-scale).
     Then per basis: T5 matmul (this chunk) + PHI2 matmul (previous chunk)
     head-accumulate in PSUM, pair-interleaved for stationary reuse.
  3. Back-transpose to c-major; residual add fused into the PSUM evacuation
     (DVE tensor_tensor); bf16 out DMA per chunk pair, host casts f32.
"""
import contextlib
import ctypes
import sys
import types

import numpy as np

for _p in ("/root/.axon_site/_ro/trn_rl_repo", "/opt/trn_rl_repo"):
    if _p not in sys.path:
        sys.path.append(_p)

B, C, N, H = 4, 512, 4096, 8
EPS = 1e-5
N_CORES = 8
NHALF = N // 2
CT = C // 128  # channel tiles
L = 128  # EMA chunk length
W = 128  # halo (q_max^128 < 1e-30 for this problem; assert at host)
NW = NHALF + W
K0 = W // L
NCH = NW // L
GSZ = 4  # chunks per stat group
R = 4  # basis rank
LAGS = 160
OUT_BF16 = True  # device emits bf16 output; host casts to f32


# ---------------------------------------------------------------------------
# axon NTFF shim (lets run_bass_kernel_spmd(trace=True) capture HW profiles)
# ---------------------------------------------------------------------------
def _install_ntff_shim():
    if "antenv.axon_hooks" in sys.modules:
        return
    holder = {"hook": None}

    def _make(so_path):
        try:
            lib = ctypes.CDLL(so_path)
        except OSError:
            return None
        if not hasattr(lib, "axon_start_nrt_profile"):
            return None
        lib.axon_start_nrt_profile.argtypes = [
            ctypes.POINTER(ctypes.c_int64),
            ctypes.c_size_t,
        ]
        lib.axon_start_nrt_profile.restype = ctypes.c_int64
        lib.axon_stop_nrt_profile.argtypes = [ctypes.c_char_p]
        lib.axon_stop_nrt_profile.restype = ctypes.c_int64

        @contextlib.contextmanager
        def _hook(output_dir, device_ids):
            import jax

            jax.devices()
            if device_ids:
                ids = (ctypes.c_int64 * len(device_ids))(*device_ids)
                rc = lib.axon_start_nrt_profile(ids, len(device_ids))
            else:
                rc = lib.axon_start_nrt_profile(None, 0)
            if rc != 0:
                raise RuntimeError(f"axon_start_nrt_profile rc={rc}")
            try:
                yield
            finally:
                n = lib.axon_stop_nrt_profile(str(output_dir).encode())
                print(f"ntff profile: {n} file(s) -> {output_dir}", file=sys.stderr)

        return _hook

    mod = types.ModuleType("antenv.axon_hooks")
    mod.set_axon_ntff_profile_hook = lambda h: holder.__setitem__("hook", h)
    mod.get_axon_ntff_profile_hook = lambda: holder["hook"]
    sys.modules["antenv.axon_hooks"] = mod
    try:
        import antenv

        antenv.axon_hooks = mod
    except ImportError:
        pass
    holder["hook"] = _make("/opt/axon/libaxon_pjrt.so")


def _split_multiwait(nc, max_waits=1):
    """This walrus build rejects >1 sync wait per instruction; split extras
    onto same-engine NoOps inserted just before (per-engine order is the
    execution order, so semantics are preserved)."""
    from concourse import mybir

    k = [0]
    for fn in nc.m.functions:
        for blk in fn.blocks:
            out = []
            for inst in blk.instructions:
                si = getattr(inst, "sync_info", None)
                if si is not None and len(si.on_wait) > max_waits:
                    waits = list(si.on_wait)
                    for w in waits[max_waits:]:
                        k[0] += 1
                        out.append(
                            mybir.InstNoOp(
                                name=f"{inst.name}-mw{k[0]}",
                                sync_info=mybir.SyncInfo(on_wait=[w], on_update=[]),
                                bass_nofuse=True,
                                engine=inst.engine,
                            )
                        )
                    inst.sync_info = mybir.SyncInfo(
                        on_wait=waits[:max_waits], on_update=list(si.on_update)
                    )
                out.append(inst)
            blk.instructions[:] = out


# ---------------------------------------------------------------------------
# program builder
# ---------------------------------------------------------------------------
def build_program():
    import concourse.bass as bass
    import concourse.tile as tile
    from concourse import mybir

    # first group = just the halo chunk so the pipeline starts fast
    stat_slices = [(0, L)]
    o = L
    while o < NW:
        w = min(GSZ * L, NW - o)
        stat_slices.append((o, w))
        o += w
    f32 = mybir.dt.float32
    bf16 = mybir.dt.bfloat16
    out_dt = bf16 if OUT_BF16 else f32
    Op = mybir.AluOpType
    Act = mybir.ActivationFunctionType

    nc = bass.Bass(
        "TRN2",
        target_bir_lowering=False,
        debug=False,
        enable_asserts=False,
        num_devices=N_CORES,
    )
    xs_d = nc.dram_tensor("xs", [C, NW], f32, kind="ExternalInput").ap()
    t5_d = nc.dram_tensor("t5", [R * 128, 128], bf16, kind="ExternalInput").ap()
    p2_d = nc.dram_tensor("phi2", [R * 128, 128], bf16, kind="ExternalInput").ap()
    w5_d = nc.dram_tensor("w5", [CT * 128, 512], bf16, kind="ExternalInput").ap()
    id_d = nc.dram_tensor("ident", [128, 128], bf16, kind="ExternalInput").ap()
    oc_d = nc.dram_tensor("onecol", [128, 2], bf16, kind="ExternalInput").ap()
    on_d = nc.dram_tensor("onesf", [128, 128], f32, kind="ExternalInput").ap()
    out_d = nc.dram_tensor("out_t", [C, NHALF], out_dt, kind="ExternalOutput").ap()

    with tile.TileContext(nc) as tc:
        with contextlib.ExitStack() as ctx:
            pers = ctx.enter_context(tc.tile_pool(name="pers", bufs=1))
            sq_pool = ctx.enter_context(tc.tile_pool(name="sqp", bufs=2))
            ps_pool = ctx.enter_context(tc.tile_pool(name="ps", bufs=1, space="PSUM"))
            st_pool = ctx.enter_context(tc.tile_pool(name="stats", bufs=2))
            xh_pool = ctx.enter_context(tc.tile_pool(name="xhp", bufs=3))
            s_pool = ctx.enter_context(tc.tile_pool(name="sp", bufs=3))
            out_pool = ctx.enter_context(tc.tile_pool(name="outp", bufs=3))
            rc_pool = ctx.enter_context(tc.tile_pool(name="rcp", bufs=3))

            # epsb memset doubles as the GpSimd Q7-IRAM warm-up (~6us,
            # overlapped with the input DMAs)
            epsb = pers.tile([128, 1], f32, tag="eps")
            nc.gpsimd.memset(epsb[:], EPS)
            # everything rides the sync HWDGE queue (FIFO): first stat group,
            # then consts, then the remaining groups
            f32r = mybir.dt.float32r
            xb = pers.tile([128, CT * NW], f32r, tag="xb")
            zc = pers.tile([128, CT * NW], bf16, tag="zc")
            xs3 = xs_d.bitcast(f32r).rearrange("(ct p) n -> p ct n", ct=CT)
            xb3 = xb[:].rearrange("p (ct n) -> p ct n", ct=CT)  # f32r view
            xbf3 = xb[:].bitcast(f32).rearrange("p (ct n) -> p ct n", ct=CT)
            zc3 = zc[:].rearrange("p (ct n) -> p ct n", ct=CT)

            def load_group(g):
                o, wd = stat_slices[g]
                nc.sync.dma_start(
                    out=xb3[:, :, o : o + wd], in_=xs3[:, :, o : o + wd]
                )

            load_group(0)
            ident = pers.tile([128, 128], bf16, tag="ident")
            nc.sync.dma_start(out=ident[:], in_=id_d)
            actwarm = pers.tile([128, 1], f32, tag="actwarm")
            nc.scalar.activation(out=actwarm[:], in_=ident[:, 0:1], func=Act.Exp)
            onecol = pers.tile([128, 2], bf16, tag="onecol")
            nc.sync.dma_start(out=onecol[:], in_=oc_d)
            ones = pers.tile([128, 128], f32r, tag="ones")
            nc.sync.dma_start(out=ones[:], in_=on_d.bitcast(f32r))
            T5 = [pers.tile([128, 128], bf16, tag=f"T{b}", name=f"T{b}") for b in range(R)]
            for b in range(R):
                nc.sync.dma_start(out=T5[b][:], in_=t5_d[b * 128 : (b + 1) * 128, :])
            P2 = [pers.tile([128, 128], bf16, tag=f"P{b}", name=f"P{b}") for b in range(R)]
            for b in range(R):
                nc.sync.dma_start(out=P2[b][:], in_=p2_d[b * 128 : (b + 1) * 128, :])
            W5 = [pers.tile([128, 512], bf16, tag=f"W5_{i}", name=f"W5_{i}") for i in range(CT)]
            for i in range(CT):
                nc.sync.dma_start(out=W5[i][:], in_=w5_d[i * 128 : (i + 1) * 128, :])
            for g in range(1, len(stat_slices)):
                load_group(g)

            rcols = {}

            def emit_stats(g):
                o, wd = stat_slices[g]
                nk = wd // L
                ps_m = ps_pool.tile([128, 512], f32, tag="misc", bufs=2)
                for ct in range(CT):
                    nc.tensor.matmul(
                        out=ps_m[:, :wd], lhsT=ones[:],
                        rhs=xb3[:, ct, o : o + wd],
                        start=(ct == 0), stop=(ct == CT - 1),
                    )
                m_rep = st_pool.tile([128, 512], bf16, tag="meanbf")
                nc.scalar.activation(out=m_rep[:, :wd], in_=ps_m[:, :wd], func=Act.Copy)
                zsq = sq_pool.tile([128, CT * 512], bf16, tag="xsq", name=f"zsq{g}")
                zsq3 = zsq[:].rearrange("p (ct n) -> p ct n", ct=CT)
                for ct in range(CT):
                    zeng = nc.vector if (g == 0 or ct < 2) else nc.gpsimd
                    zeng.tensor_tensor(
                        out=zc3[:, ct, o : o + wd], in0=xbf3[:, ct, o : o + wd],
                        in1=m_rep[:, :wd], op=Op.subtract,
                    )
                    eng = nc.vector if (g == 0 or ct % 2 == 0) else nc.gpsimd
                    eng.tensor_tensor(
                        out=zsq3[:, ct, :wd], in0=zc3[:, ct, o : o + wd],
                        in1=zc3[:, ct, o : o + wd], op=Op.mult,
                    )
                scol_ps = ps_pool.tile([128, nk], f32, tag="misc", bufs=2)
                for kk in range(nk):
                    for ct in range(CT):
                        nc.tensor.matmul(
                            out=scol_ps[:, kk : kk + 1],
                            lhsT=zsq3[:, ct, kk * L : (kk + 1) * L],
                            rhs=onecol[:, 1:2],
                            start=(ct == 0), stop=(ct == CT - 1),
                        )
                var = st_pool.tile([128, nk], f32, tag="varc")
                nc.vector.tensor_scalar(
                    out=var[:], in0=scol_ps[:], scalar1=1.0, scalar2=None,
                    op0=Op.mult,
                )
                lnv = st_pool.tile([128, nk], f32, tag="lnvc")
                nc.scalar.activation(out=lnv[:], in_=var[:], func=Act.Ln, bias=epsb[:])
                rc = rc_pool.tile([128, nk], f32, tag="rcol", name=f"rcol{g}")
                nc.scalar.activation(out=rc[:], in_=lnv[:], func=Act.Exp, scale=-0.5)
                rcols[g] = rc

            def grp(k):
                return 0 if k == 0 else (k - 1) // GSZ + 1

            def r_col(k):
                g = grp(k)
                kk = 0 if k == 0 else (k - 1) % GSZ
                return rcols[g][:, kk : kk + 1]

            def zc_slice(k, dt):
                return zc3[:, dt, k * L : (k + 1) * L]

            def make_xh(k):
                """scaled transposes: xh cols = dt*512 + b*128 + c.
                One LDW per dtile."""
                xh = xh_pool.tile([128, 4 * 512], bf16, tag="xh")
                for dp in range(2):
                    sp = ps_pool.tile([128, 1024], f32, tag="xps", bufs=2,
                                      name=f"xps{k}_{dp}")
                    for dd in range(2):
                        dt = dp * 2 + dd
                        nc.tensor.matmul(
                            out=sp[:, dd * 512 : (dd + 1) * 512],
                            lhsT=zc_slice(k, dt), rhs=W5[dt][:],
                            start=True, stop=True,
                        )
                    dst = xh[:, dp * 1024 : (dp + 1) * 1024]
                    if dp == 0:
                        nc.vector.tensor_scalar(
                            out=dst, in0=sp[:], scalar1=r_col(k), scalar2=None,
                            op0=Op.mult,
                        )
                    else:
                        nc.scalar.activation(
                            out=dst, in_=sp[:], func=Act.Copy, scale=r_col(k)
                        )
                return xh[:].rearrange("p (dt b c) -> p dt b c", dt=CT, b=4)

            def rhs_b(xh4, b):
                return xh4[:, :, b, :]

            def chunk_tail(k, ema_ps, ot, half):
                s_sb = s_pool.tile([128, 512], bf16, tag="ssb")
                nc.scalar.activation(out=s_sb[:], in_=ema_ps[:], func=Act.Copy)
                t_ps = ps_pool.tile([128, 512], f32, tag="ema", bufs=2)
                for dt in range(CT):
                    nc.tensor.matmul(
                        out=t_ps[:, dt * 128 : (dt + 1) * 128],
                        lhsT=s_sb[:, dt * 128 : (dt + 1) * 128], rhs=ident[:],
                        start=True, stop=True,
                    )
                resid = xbf3[:, :, k * L : (k + 1) * L]
                ot3 = ot[:].rearrange("p (dt i) -> p dt i", dt=CT)
                nc.vector.tensor_tensor(
                    out=ot3[:, :, half * L : (half + 1) * L],
                    in0=t_ps[:].rearrange("p (dt i) -> p dt i", dt=CT),
                    in1=resid, op=Op.add,
                )

            # ---- emission: stats groups interleaved with chunk pairs ----
            ks = list(range(K0, NCH))
            pairs = [ks[i : i + 2] for i in range(0, len(ks), 2)]
            emitted = set()

            def need_group(g):
                if g not in emitted and g < len(stat_slices):
                    emitted.add(g)
                    emit_stats(g)

            need_group(0)
            prev = None
            for k in range(K0):  # halo chunks: correction source only
                prev = make_xh(k)

            for pair in pairs:
                for k in pair:
                    need_group(grp(k))
                xhs, psums = [], []
                for k in pair:
                    xhs.append(make_xh(k))
                prevs = [prev, xhs[0]]
                for b in range(R):  # this-chunk triangular, pair-interleaved
                    for i, k in enumerate(pair):
                        if b == 0:
                            psums.append(ps_pool.tile([128, 512], f32, tag="ema",
                                                      bufs=2, name=f"emaps{k}"))
                        nc.tensor.matmul(
                            out=psums[i][:], lhsT=T5[b][:], rhs=rhs_b(xhs[i], b),
                            start=(b == 0), stop=False,
                        )
                for b in range(R):  # previous-chunk correction
                    for i, k in enumerate(pair):
                        nc.tensor.matmul(
                            out=psums[i][:], lhsT=P2[b][:], rhs=rhs_b(prevs[i], b),
                            start=False, stop=(b == R - 1),
                        )
                prev = xhs[-1]
                ot = out_pool.tile([128, CT * 2 * L], out_dt, tag="out")
                for i, k in enumerate(pair):
                    chunk_tail(k, psums[i], ot, i)
                ko = pair[0] - K0
                nc.sync.dma_start(
                    out=out_d.rearrange("(dt p) n -> p dt n", dt=CT)[
                        :, :, ko * L : (ko + 2) * L
                    ],
                    in_=ot[:].rearrange("p (dt i) -> p dt i", dt=CT),
                )
                need_group(grp(pair[1] + 2))  # prefetch: emitted after PE work
    return nc


def _host_params(ln_gamma, ln_beta, expansion, reduction, alphas, dampen_factors):
    import ml_dtypes

    a = 1.0 / (1.0 + np.exp(-alphas.astype(np.float64)))
    q = (1.0 - a) / (1.0 + np.exp(-dampen_factors.astype(np.float64)))
    qmax = float(q.max())
    assert qmax**W < 1e-8, f"halo W={W} too small for qmax={qmax}"
    rho = (  # WITHOUT a_h: amplitude lives in the kernel matrix M
        expansion.astype(np.float64)
        * reduction.astype(np.float64)
        * ln_gamma.astype(np.float64)[None, :]
    )  # [H, C]
    lag = np.arange(LAGS)
    M = a[:, None] * (q[:, None] ** lag[None, :])  # [H, LAGS]
    U, S, Vt = np.linalg.svd(M, full_matrices=False)
    beta = U[:, :R] * S[:R]  # [H, R]
    phi = Vt[:R]  # [R, LAGS]
    cb = np.einsum("hd,hb->bd", rho, beta)  # [R, C]

    bf = ml_dtypes.bfloat16
    ii, jj = np.meshgrid(np.arange(L), np.arange(L), indexing="ij")
    t5 = np.zeros((R * 128, 128), bf)
    p2 = np.zeros((R * 128, 128), bf)
    for b in range(R):
        lagm = ii - jj
        Tb = np.where(lagm >= 0, phi[b][np.clip(lagm, 0, LAGS - 1)], 0.0)
        t5[b * 128 : (b + 1) * 128, :] = Tb.T.astype(bf)  # lhsT[j,i]
        lag2 = ii + L - jj  # lag from previous chunk, in [1, 255]
        P2b = np.where(lag2 < LAGS, phi[b][np.clip(lag2, 0, LAGS - 1)], 0.0)
        p2[b * 128 : (b + 1) * 128, :] = P2b.T.astype(bf)
    w5 = np.zeros((CT * 128, 512), bf)
    for dt in range(CT):
        blk = np.zeros((128, 512))
        for b in range(R):
            blk[:, b * 128 : (b + 1) * 128] = np.diag(cb[b, dt * 128 : (dt + 1) * 128])
        w5[dt * 128 : (dt + 1) * 128, :] = blk.astype(bf)
    ident = np.eye(128, dtype=bf)
    onecol = np.zeros((128, 2), bf)
    onecol[:, 0] = 1.0 / 128.0
    onecol[:, 1] = 1.0 / C
    onesf = np.full((128, 128), 1.0 / C, np.float32)
    consts = dict(t5=t5, phi2=p2, w5=w5, ident=ident, onecol=onecol, onesf=onesf)
    return a, q, consts


def _beta_term(ln_beta, expansion, reduction, a, q):
    if not np.any(ln_beta):
        return None
    n_idx = np.arange(N, dtype=np.float64)
    Cn = a[:, None] * (1.0 - q[:, None] ** (n_idx[None, :] + 1.0)) / (1.0 - q[:, None])
    w = (
        expansion.astype(np.float64)
        * reduction.astype(np.float64)
        * ln_beta.astype(np.float64)[None, :]
    )
    return np.einsum("hc,hn->cn", w, Cn).astype(np.float32)


def _make_in_maps(x, consts):
    in_maps = []
    for core in range(N_CORES):
        b, half = divmod(core, 2)
        xs = np.zeros((C, NW), np.float32)
        s = half * NHALF - W
        if s < 0:
            xs[:, W:] = x[b, :, :NHALF]
        else:
            xs[:] = x[b, :, s : s + NW]
        in_maps.append(dict(consts, xs=xs))
    return in_maps


def kernel(x, ln_gamma, ln_beta, expansion, reduction, alphas, dampen_factors,
           trace=False):
    _install_ntff_shim()
    from concourse.bass_utils import run_bass_kernel_spmd
    from concourse.bass_interp import get_hw_module

    x = np.asarray(x, np.float32)
    a, q, consts = _host_params(
        np.asarray(ln_gamma), np.asarray(ln_beta), np.asarray(expansion),
        np.asarray(reduction), np.asarray(alphas), np.asarray(dampen_factors),
    )
    nc = build_program()
    _split_multiwait(nc)
    nc.m = get_hw_module(nc.m)

    in_maps = _make_in_maps(x, consts)
    res = run_bass_kernel_spmd(
        nc, in_maps, core_ids=list(range(N_CORES)), trace=trace
    )

    out = np.empty((B, C, N), np.float32)
    for core in range(N_CORES):
        b, half = divmod(core, 2)
        out[b, :, half * NHALF : (half + 1) * NHALF] = np.asarray(
            res.results[core]["out_t"], np.float32
        )
    bt = _beta_term(
        np.asarray(ln_beta), np.asarray(expansion), np.asarray(reduction), a, q
    )
    if bt is not None:
        out += bt[None]
    if trace:
        kernel.last_results = res
    return out


# revision 17
# speedup vs baseline: 1.0198x; 1.0198x over previous
"""MultiHeadEMABlock Trainium2 kernel (8-core SPMD, bass/Tile) — v3.

Math (reference):
  h = LayerNorm_c(x[b,c,n] over c) * gamma + beta          (per (b,n))
  xe[b,n,h,d] = h[b,n,d] * expansion[h,d]
  y = causal damped EMA along n: y[t] = a_h*sum_{s<=t} q_h^{t-s} xe[s]
  out[b,d,n] = sum_h y[b,n,h,d]*reduction[h,d] + x

Identities used:
  - Per-(h,d) scales commute with the EMA: out = x + sum_h rho_h[d]*S_h[d,n].
  - RANK-5 BASIS: the 8 exponential kernels a_h q_h^l (l in [0,160)) lie in a
    rank-5 subspace (SVD, max per-head rel err 1.9e-4). With basis phi_b and
    per-channel coefficients c_b[d] = sum_h rho_h[d] beta[h,b], the head sum
    collapses to 5 "basis heads":
      out_ema[d,t] = sum_b c_b[d] * (phi_b (*) z)[d,t]
  - q_max^128 ~ 1e-31, so each 128-chunk needs only its own + the previous
    chunk as history: cross-chunk carry state is replaced by a second
    triangular matmul (PHI2) against the PREVIOUS chunk's transposed inputs.
    No serial carry chain at all.
  - rstd is position-wise so it commutes with the c->n transpose: applied as
    a per-partition scale while evacuating the transposed PSUM.
  - beta(LN) contributes a data-independent term added on host (exact).

Sharding: 8 cores = 4 batches x 2 sequence halves, W=128 left halo.

Device algorithm (per core, c-major [channel x n] base layout):
  1. x loaded via SWDGE cast-DMA (f32->bf16). Mean via ones-matmul
     (replicated); zc = xb - mean on GpSimd. Position-column stats via tiny
     N=1 matmuls; r_col = exp(-.5 ln(var+eps)) on ACT over [128, nk] tiles.
  2. Per chunk: one LDW per dtile serves two scale+transpose matmuls (basis
     0-3 diag rhs N=512, basis-4 plane N=128); PSUM evacuated with the
     per-partition r_col scale fused (DVE tensor_scalar / ACT act-scale).
     Then per basis: T5 matmul (this chunk) + PHI2 matmul (previous chunk)
     head-accumulate in PSUM, pair-interleaved for stationary reuse.
  3. Back-transpose to c-major; residual add fused into the PSUM evacuation
     (DVE tensor_tensor); bf16 out DMA per chunk pair, host casts f32.
"""
import contextlib
import ctypes
import sys
import types

import numpy as np

for _p in ("/root/.axon_site/_ro/trn_rl_repo", "/opt/trn_rl_repo"):
    if _p not in sys.path:
        sys.path.append(_p)

B, C, N, H = 4, 512, 4096, 8
EPS = 1e-5
N_CORES = 8
NHALF = N // 2
CT = C // 128  # channel tiles
L = 128  # EMA chunk length
W = 128  # halo (q_max^128 < 1e-30 for this problem; assert at host)
NW = NHALF + W
K0 = W // L
NCH = NW // L
GSZ = 4  # chunks per stat group
R = 4  # basis rank
LAGS = 160
OUT_BF16 = True  # device emits bf16 output; host casts to f32


# ---------------------------------------------------------------------------
# axon NTFF shim (lets run_bass_kernel_spmd(trace=True) capture HW profiles)
# ---------------------------------------------------------------------------
def _install_ntff_shim():
    if "antenv.axon_hooks" in sys.modules:
        return
    holder = {"hook": None}

    def _make(so_path):
        try:
            lib = ctypes.CDLL(so_path)
        except OSError:
            return None
        if not hasattr(lib, "axon_start_nrt_profile"):
            return None
        lib.axon_start_nrt_profile.argtypes = [
            ctypes.POINTER(ctypes.c_int64),
            ctypes.c_size_t,
        ]
        lib.axon_start_nrt_profile.restype = ctypes.c_int64
        lib.axon_stop_nrt_profile.argtypes = [ctypes.c_char_p]
        lib.axon_stop_nrt_profile.restype = ctypes.c_int64

        @contextlib.contextmanager
        def _hook(output_dir, device_ids):
            import jax

            jax.devices()
            if device_ids:
                ids = (ctypes.c_int64 * len(device_ids))(*device_ids)
                rc = lib.axon_start_nrt_profile(ids, len(device_ids))
            else:
                rc = lib.axon_start_nrt_profile(None, 0)
            if rc != 0:
                raise RuntimeError(f"axon_start_nrt_profile rc={rc}")
            try:
                yield
            finally:
                n = lib.axon_stop_nrt_profile(str(output_dir).encode())
                print(f"ntff profile: {n} file(s) -> {output_dir}", file=sys.stderr)

        return _hook

    mod = types.ModuleType("antenv.axon_hooks")
    mod.set_axon_ntff_profile_hook = lambda h: holder.__setitem__("hook", h)
    mod.get_axon_ntff_profile_hook = lambda: holder["hook"]
    sys.modules["antenv.axon_hooks"] = mod
    try:
        import antenv

        antenv.axon_hooks = mod
    except ImportError:
        pass
    holder["hook"] = _make("/opt/axon/libaxon_pjrt.so")


def _split_multiwait(nc, max_waits=1):
    """This walrus build rejects >1 sync wait per instruction; split extras
    onto same-engine NoOps inserted just before (per-engine order is the
    execution order, so semantics are preserved)."""
    from concourse import mybir

    k = [0]
    for fn in nc.m.functions:
        for blk in fn.blocks:
            out = []
            for inst in blk.instructions:
                si = getattr(inst, "sync_info", None)
                if si is not None and len(si.on_wait) > max_waits:
                    waits = list(si.on_wait)
                    for w in waits[max_waits:]:
                        k[0] += 1
                        out.append(
                            mybir.InstNoOp(
                                name=f"{inst.name}-mw{k[0]}",
                                sync_info=mybir.SyncInfo(on_wait=[w], on_update=[]),
                                bass_nofuse=True,
                                engine=inst.engine,
                            )
                        )
                    inst.sync_info = mybir.SyncInfo(
                        on_wait=waits[:max_waits], on_update=list(si.on_update)
                    )
                out.append(inst)
            blk.instructions[:] = out


# ---------------------------------------------------------------------------
# program builder
# ---------------------------------------------------------------------------
def build_program():
    import concourse.bass as bass
    import concourse.tile as tile
    from concourse import mybir

    # first group = just the halo chunk so the pipeline starts fast
    stat_slices = [(0, L)]
    o = L
    while o < NW:
        w = min(GSZ * L, NW - o)
        stat_slices.append((o, w))
        o += w
    f32 = mybir.dt.float32
    bf16 = mybir.dt.bfloat16
    out_dt = bf16 if OUT_BF16 else f32
    Op = mybir.AluOpType
    Act = mybir.ActivationFunctionType

    nc = bass.Bass(
        "TRN2",
        target_bir_lowering=False,
        debug=False,
        enable_asserts=False,
        num_devices=N_CORES,
    )
    xs_d = nc.dram_tensor("xs", [C, NW], f32, kind="ExternalInput").ap()
    cp_d = nc.dram_tensor("cpack", [128, 3460], bf16, kind="ExternalInput").ap()
    on_d = nc.dram_tensor("onesf", [128, 128], f32, kind="ExternalInput").ap()
    out_d = nc.dram_tensor("out_t", [C, NHALF], out_dt, kind="ExternalOutput").ap()

    with tile.TileContext(nc) as tc:
        with contextlib.ExitStack() as ctx:
            pers = ctx.enter_context(tc.tile_pool(name="pers", bufs=1))
            sq_pool = ctx.enter_context(tc.tile_pool(name="sqp", bufs=2))
            ps_pool = ctx.enter_context(tc.tile_pool(name="ps", bufs=1, space="PSUM"))
            st_pool = ctx.enter_context(tc.tile_pool(name="stats", bufs=2))
            xh_pool = ctx.enter_context(tc.tile_pool(name="xhp", bufs=3))
            s_pool = ctx.enter_context(tc.tile_pool(name="sp", bufs=3))
            out_pool = ctx.enter_context(tc.tile_pool(name="outp", bufs=3))
            rc_pool = ctx.enter_context(tc.tile_pool(name="rcp", bufs=3))

            # one packed const DMA (each dma_start costs ~1us of queue
            # time, so batch everything); f32 regions ride as bf16 bitcasts
            f32r = mybir.dt.float32r
            cpack = pers.tile([128, 3460], bf16, tag="cpack")
            xb = pers.tile([128, CT * NW], f32r, tag="xb")
            zc = pers.tile([128, CT * NW], bf16, tag="zc")
            xs3 = xs_d.bitcast(f32r).rearrange("(ct p) n -> p ct n", ct=CT)
            xb3 = xb[:].rearrange("p (ct n) -> p ct n", ct=CT)  # f32r view
            xbf3 = xb[:].bitcast(f32).rearrange("p (ct n) -> p ct n", ct=CT)
            zc3 = zc[:].rearrange("p (ct n) -> p ct n", ct=CT)

            def load_group(g, queue):
                o, wd = stat_slices[g]
                queue.dma_start(
                    out=xb3[:, :, o : o + wd], in_=xs3[:, :, o : o + wd]
                )

            # sync queue: g0, consts, g2, g4; scalar queue: g1, g3
            load_group(0, nc.sync)
            nc.sync.dma_start(out=cpack[:], in_=cp_d)
            ones_t = pers.tile([128, 128], f32r, tag="ones")
            nc.sync.dma_start(out=ones_t[:], in_=on_d.bitcast(f32r))
            load_group(1, nc.scalar)
            load_group(2, nc.sync)
            load_group(3, nc.scalar)
            load_group(4, nc.sync)
            T5 = [cpack[:, b * 128 : (b + 1) * 128] for b in range(R)]
            P2 = [cpack[:, 512 + b * 128 : 512 + (b + 1) * 128] for b in range(R)]
            W5 = [cpack[:, 1024 + i * 512 : 1024 + (i + 1) * 512] for i in range(CT)]
            ident = cpack[:, 3072:3200]
            onecol = cpack[:, 3200:3202]
            ones = ones_t[:]
            epsb = cpack[:, 3458:3460].bitcast(f32)
            actwarm = pers.tile([128, 1], f32, tag="actwarm")
            nc.scalar.activation(out=actwarm[:], in_=ident[:, 0:1], func=Act.Exp)

            rcols = {}

            def emit_stats(g):
                o, wd = stat_slices[g]
                nk = wd // L
                ps_m = ps_pool.tile([128, 512], f32, tag="misc", bufs=2)
                for ct in range(CT):
                    nc.tensor.matmul(
                        out=ps_m[:, :wd], lhsT=ones,
                        rhs=xb3[:, ct, o : o + wd],
                        start=(ct == 0), stop=(ct == CT - 1),
                    )
                m_rep = st_pool.tile([128, 512], bf16, tag="meanbf")
                nc.scalar.activation(out=m_rep[:, :wd], in_=ps_m[:, :wd], func=Act.Copy)
                zsq = sq_pool.tile([128, CT * 512], bf16, tag="xsq", name=f"zsq{g}")
                zsq3 = zsq[:].rearrange("p (ct n) -> p ct n", ct=CT)
                for ct in range(CT):
                    zeng = nc.vector if (g == 0 or ct < 2) else nc.gpsimd
                    zeng.tensor_tensor(
                        out=zc3[:, ct, o : o + wd], in0=xbf3[:, ct, o : o + wd],
                        in1=m_rep[:, :wd], op=Op.subtract,
                    )
                    eng = nc.vector if (g == 0 or ct % 2 == 0) else nc.gpsimd
                    eng.tensor_tensor(
                        out=zsq3[:, ct, :wd], in0=zc3[:, ct, o : o + wd],
                        in1=zc3[:, ct, o : o + wd], op=Op.mult,
                    )
                scol_ps = ps_pool.tile([128, nk], f32, tag="misc", bufs=2)
                for kk in range(nk):
                    for ct in range(CT):
                        nc.tensor.matmul(
                            out=scol_ps[:, kk : kk + 1],
                            lhsT=zsq3[:, ct, kk * L : (kk + 1) * L],
                            rhs=onecol[:, 1:2],
                            start=(ct == 0), stop=(ct == CT - 1),
                        )
                var = st_pool.tile([128, nk], f32, tag="varc")
                nc.vector.tensor_scalar(
                    out=var[:], in0=scol_ps[:], scalar1=1.0, scalar2=None,
                    op0=Op.mult,
                )
                lnv = st_pool.tile([128, nk], f32, tag="lnvc")
                nc.scalar.activation(out=lnv[:], in_=var[:], func=Act.Ln, bias=epsb)
                rc = rc_pool.tile([128, nk], f32, tag="rcol", name=f"rcol{g}")
                nc.scalar.activation(out=rc[:], in_=lnv[:], func=Act.Exp, scale=-0.5)
                rcols[g] = rc

            def grp(k):
                return 0 if k == 0 else (k - 1) // GSZ + 1

            def r_col(k):
                g = grp(k)
                kk = 0 if k == 0 else (k - 1) % GSZ
                return rcols[g][:, kk : kk + 1]

            def zc_slice(k, dt):
                return zc3[:, dt, k * L : (k + 1) * L]

            def make_xh(k):
                """scaled transposes: xh cols = dt*512 + b*128 + c.
                One LDW per dtile."""
                xh = xh_pool.tile([128, 4 * 512], bf16, tag="xh")
                for dp in range(2):
                    sp = ps_pool.tile([128, 1024], f32, tag="xps", bufs=2,
                                      name=f"xps{k}_{dp}")
                    for dd in range(2):
                        dt = dp * 2 + dd
                        nc.tensor.matmul(
                            out=sp[:, dd * 512 : (dd + 1) * 512],
                            lhsT=zc_slice(k, dt), rhs=W5[dt],
                            start=True, stop=True,
                        )
                    dst = xh[:, dp * 1024 : (dp + 1) * 1024]
                    if dp == 0:
                        nc.vector.tensor_scalar(
                            out=dst, in0=sp[:], scalar1=r_col(k), scalar2=None,
                            op0=Op.mult,
                        )
                    else:
                        nc.scalar.activation(
                            out=dst, in_=sp[:], func=Act.Copy, scale=r_col(k)
                        )
                return xh[:].rearrange("p (dt b c) -> p dt b c", dt=CT, b=4)

            def rhs_b(xh4, b):
                return xh4[:, :, b, :]

            def chunk_tail(k, ema_ps, ot, half):
                s_sb = s_pool.tile([128, 512], bf16, tag="ssb")
                nc.scalar.activation(out=s_sb[:], in_=ema_ps[:], func=Act.Copy)
                t_ps = ps_pool.tile([128, 512], f32, tag="ema", bufs=2)
                for dt in range(CT):
                    nc.tensor.matmul(
                        out=t_ps[:, dt * 128 : (dt + 1) * 128],
                        lhsT=s_sb[:, dt * 128 : (dt + 1) * 128], rhs=ident,
                        start=True, stop=True,
                    )
                resid = xbf3[:, :, k * L : (k + 1) * L]
                ot3 = ot[:].rearrange("p (dt i) -> p dt i", dt=CT)
                nc.vector.tensor_tensor(
                    out=ot3[:, :, half * L : (half + 1) * L],
                    in0=t_ps[:].rearrange("p (dt i) -> p dt i", dt=CT),
                    in1=resid, op=Op.add,
                )

            # ---- emission: stats groups interleaved with chunk pairs ----
            ks = list(range(K0, NCH))
            pairs = [ks[i : i + 2] for i in range(0, len(ks), 2)]
            emitted = set()

            def need_group(g):
                if g not in emitted and g < len(stat_slices):
                    emitted.add(g)
                    emit_stats(g)

            need_group(0)
            prev = None
            for k in range(K0):  # halo chunks: correction source only
                prev = make_xh(k)

            for pair in pairs:
                for k in pair:
                    need_group(grp(k))
                xhs, psums = [], []
                for k in pair:
                    xhs.append(make_xh(k))
                prevs = [prev, xhs[0]]
                for b in range(R):  # this-chunk triangular, pair-interleaved
                    for i, k in enumerate(pair):
                        if b == 0:
                            psums.append(ps_pool.tile([128, 512], f32, tag="ema",
                                                      bufs=2, name=f"emaps{k}"))
                        nc.tensor.matmul(
                            out=psums[i][:], lhsT=T5[b], rhs=rhs_b(xhs[i], b),
                            start=(b == 0), stop=False,
                        )
                for b in range(R):  # previous-chunk correction
                    for i, k in enumerate(pair):
                        nc.tensor.matmul(
                            out=psums[i][:], lhsT=P2[b], rhs=rhs_b(prevs[i], b),
                            start=False, stop=(b == R - 1),
                        )
                prev = xhs[-1]
                ot = out_pool.tile([128, CT * 2 * L], out_dt, tag="out")
                for i, k in enumerate(pair):
                    chunk_tail(k, psums[i], ot, i)
                ko = pair[0] - K0
                nc.sync.dma_start(
                    out=out_d.rearrange("(dt p) n -> p dt n", dt=CT)[
                        :, :, ko * L : (ko + 2) * L
                    ],
                    in_=ot[:].rearrange("p (dt i) -> p dt i", dt=CT),
                )
                need_group(grp(pair[1] + 2))  # prefetch: emitted after PE work
    return nc


def _host_params(ln_gamma, ln_beta, expansion, reduction, alphas, dampen_factors):
    import ml_dtypes

    a = 1.0 / (1.0 + np.exp(-alphas.astype(np.float64)))
    q = (1.0 - a) / (1.0 + np.exp(-dampen_factors.astype(np.float64)))
    qmax = float(q.max())
    assert qmax**W < 1e-8, f"halo W={W} too small for qmax={qmax}"
    rho = (  # WITHOUT a_h: amplitude lives in the kernel matrix M
        expansion.astype(np.float64)
        * reduction.astype(np.float64)
        * ln_gamma.astype(np.float64)[None, :]
    )  # [H, C]
    lag = np.arange(LAGS)
    M = a[:, None] * (q[:, None] ** lag[None, :])  # [H, LAGS]
    U, S, Vt = np.linalg.svd(M, full_matrices=False)
    beta = U[:, :R] * S[:R]  # [H, R]
    phi = Vt[:R]  # [R, LAGS]
    cb = np.einsum("hd,hb->bd", rho, beta)  # [R, C]

    bf = ml_dtypes.bfloat16
    ii, jj = np.meshgrid(np.arange(L), np.arange(L), indexing="ij")
    t5 = np.zeros((R * 128, 128), bf)
    p2 = np.zeros((R * 128, 128), bf)
    for b in range(R):
        lagm = ii - jj
        Tb = np.where(lagm >= 0, phi[b][np.clip(lagm, 0, LAGS - 1)], 0.0)
        t5[b * 128 : (b + 1) * 128, :] = Tb.T.astype(bf)  # lhsT[j,i]
        lag2 = ii + L - jj  # lag from previous chunk, in [1, 255]
        P2b = np.where(lag2 < LAGS, phi[b][np.clip(lag2, 0, LAGS - 1)], 0.0)
        p2[b * 128 : (b + 1) * 128, :] = P2b.T.astype(bf)
    w5 = np.zeros((CT * 128, 512), bf)
    for dt in range(CT):
        blk = np.zeros((128, 512))
        for b in range(R):
            blk[:, b * 128 : (b + 1) * 128] = np.diag(cb[b, dt * 128 : (dt + 1) * 128])
        w5[dt * 128 : (dt + 1) * 128, :] = blk.astype(bf)
    ident = np.eye(128, dtype=bf)
    onecol = np.zeros((128, 2), bf)
    onecol[:, 0] = 1.0 / 128.0
    onecol[:, 1] = 1.0 / C
    onesf = np.full((128, 128), 1.0 / C, np.float32)
    epsf = np.full((128, 1), EPS, np.float32)
    cpack = np.zeros((128, 3460), bf)
    for b in range(R):
        cpack[:, b * 128 : (b + 1) * 128] = t5[b * 128 : (b + 1) * 128, :]
        cpack[:, 512 + b * 128 : 512 + (b + 1) * 128] = p2[b * 128 : (b + 1) * 128, :]
    for dt in range(CT):
        cpack[:, 1024 + dt * 512 : 1024 + (dt + 1) * 512] = w5[dt * 128 : (dt + 1) * 128, :]
    cpack[:, 3072:3200] = ident
    cpack[:, 3200:3202] = onecol
    cpack[:, 3202:3458] = onesf.view(bf)
    cpack[:, 3458:3460] = epsf.view(bf)
    consts = dict(cpack=cpack, onesf=onesf)
    return a, q, consts


def _beta_term(ln_beta, expansion, reduction, a, q):
    if not np.any(ln_beta):
        return None
    n_idx = np.arange(N, dtype=np.float64)
    Cn = a[:, None] * (1.0 - q[:, None] ** (n_idx[None, :] + 1.0)) / (1.0 - q[:, None])
    w = (
        expansion.astype(np.float64)
        * reduction.astype(np.float64)
        * ln_beta.astype(np.float64)[None, :]
    )
    return np.einsum("hc,hn->cn", w, Cn).astype(np.float32)


def _make_in_maps(x, consts):
    in_maps = []
    for core in range(N_CORES):
        b, half = divmod(core, 2)
        xs = np.zeros((C, NW), np.float32)
        s = half * NHALF - W
        if s < 0:
            xs[:, W:] = x[b, :, :NHALF]
        else:
            xs[:] = x[b, :, s : s + NW]
        in_maps.append(dict(consts, xs=xs))
    return in_maps


def kernel(x, ln_gamma, ln_beta, expansion, reduction, alphas, dampen_factors,
           trace=False):
    _install_ntff_shim()
    from concourse.bass_utils import run_bass_kernel_spmd
    from concourse.bass_interp import get_hw_module

    x = np.asarray(x, np.float32)
    a, q, consts = _host_params(
        np.asarray(ln_gamma), np.asarray(ln_beta), np.asarray(expansion),
        np.asarray(reduction), np.asarray(alphas), np.asarray(dampen_factors),
    )
    nc = build_program()
    _split_multiwait(nc)
    nc.m = get_hw_module(nc.m)

    in_maps = _make_in_maps(x, consts)
    res = run_bass_kernel_spmd(
        nc, in_maps, core_ids=list(range(N_CORES)), trace=trace
    )

    out = np.empty((B, C, N), np.float32)
    for core in range(N_CORES):
        b, half = divmod(core, 2)
        out[b, :, half * NHALF : (half + 1) * NHALF] = np.asarray(
            res.results[core]["out_t"], np.float32
        )
    bt = _beta_term(
        np.asarray(ln_beta), np.asarray(expansion), np.asarray(reduction), a, q
    )
    if bt is not None:
        out += bt[None]
    if trace:
        kernel.last_results = res
    return out


# revision 18
# speedup vs baseline: 1.0387x; 1.0185x over previous
"""MultiHeadEMABlock Trainium2 kernel (8-core SPMD, bass/Tile) — v3.

Math (reference):
  h = LayerNorm_c(x[b,c,n] over c) * gamma + beta          (per (b,n))
  xe[b,n,h,d] = h[b,n,d] * expansion[h,d]
  y = causal damped EMA along n: y[t] = a_h*sum_{s<=t} q_h^{t-s} xe[s]
  out[b,d,n] = sum_h y[b,n,h,d]*reduction[h,d] + x

Identities used:
  - Per-(h,d) scales commute with the EMA: out = x + sum_h rho_h[d]*S_h[d,n].
  - RANK-5 BASIS: the 8 exponential kernels a_h q_h^l (l in [0,160)) lie in a
    rank-5 subspace (SVD, max per-head rel err 1.9e-4). With basis phi_b and
    per-channel coefficients c_b[d] = sum_h rho_h[d] beta[h,b], the head sum
    collapses to 5 "basis heads":
      out_ema[d,t] = sum_b c_b[d] * (phi_b (*) z)[d,t]
  - q_max^128 ~ 1e-31, so each 128-chunk needs only its own + the previous
    chunk as history: cross-chunk carry state is replaced by a second
    triangular matmul (PHI2) against the PREVIOUS chunk's transposed inputs.
    No serial carry chain at all.
  - rstd is position-wise so it commutes with the c->n transpose: applied as
    a per-partition scale while evacuating the transposed PSUM.
  - beta(LN) contributes a data-independent term added on host (exact).

Sharding: 8 cores = 4 batches x 2 sequence halves, W=128 left halo.

Device algorithm (per core, c-major [channel x n] base layout):
  1. x loaded via SWDGE cast-DMA (f32->bf16). Mean via ones-matmul
     (replicated); zc = xb - mean on GpSimd. Position-column stats via tiny
     N=1 matmuls; r_col = exp(-.5 ln(var+eps)) on ACT over [128, nk] tiles.
  2. Per chunk: one LDW per dtile serves two scale+transpose matmuls (basis
     0-3 diag rhs N=512, basis-4 plane N=128); PSUM evacuated with the
     per-partition r_col scale fused (DVE tensor_scalar / ACT act-scale).
     Then per basis: T5 matmul (this chunk) + PHI2 matmul (previous chunk)
     head-accumulate in PSUM, pair-interleaved for stationary reuse.
  3. Back-transpose to c-major; residual add fused into the PSUM evacuation
     (DVE tensor_tensor); bf16 out DMA per chunk pair, host casts f32.
"""
import contextlib
import ctypes
import sys
import types

import numpy as np

for _p in ("/root/.axon_site/_ro/trn_rl_repo", "/opt/trn_rl_repo"):
    if _p not in sys.path:
        sys.path.append(_p)

B, C, N, H = 4, 512, 4096, 8
EPS = 1e-5
N_CORES = 8
NHALF = N // 2
CT = C // 128  # channel tiles
L = 128  # EMA chunk length
W = 128  # halo (q_max^128 < 1e-30 for this problem; assert at host)
NW = NHALF + W
K0 = W // L
NCH = NW // L
GSZ = 4  # chunks per stat group
R = 4  # basis rank
LAGS = 160
OUT_BF16 = True  # device emits bf16 output; host casts to f32


# ---------------------------------------------------------------------------
# axon NTFF shim (lets run_bass_kernel_spmd(trace=True) capture HW profiles)
# ---------------------------------------------------------------------------
def _install_ntff_shim():
    if "antenv.axon_hooks" in sys.modules:
        return
    holder = {"hook": None}

    def _make(so_path):
        try:
            lib = ctypes.CDLL(so_path)
        except OSError:
            return None
        if not hasattr(lib, "axon_start_nrt_profile"):
            return None
        lib.axon_start_nrt_profile.argtypes = [
            ctypes.POINTER(ctypes.c_int64),
            ctypes.c_size_t,
        ]
        lib.axon_start_nrt_profile.restype = ctypes.c_int64
        lib.axon_stop_nrt_profile.argtypes = [ctypes.c_char_p]
        lib.axon_stop_nrt_profile.restype = ctypes.c_int64

        @contextlib.contextmanager
        def _hook(output_dir, device_ids):
            import jax

            jax.devices()
            if device_ids:
                ids = (ctypes.c_int64 * len(device_ids))(*device_ids)
                rc = lib.axon_start_nrt_profile(ids, len(device_ids))
            else:
                rc = lib.axon_start_nrt_profile(None, 0)
            if rc != 0:
                raise RuntimeError(f"axon_start_nrt_profile rc={rc}")
            try:
                yield
            finally:
                n = lib.axon_stop_nrt_profile(str(output_dir).encode())
                print(f"ntff profile: {n} file(s) -> {output_dir}", file=sys.stderr)

        return _hook

    mod = types.ModuleType("antenv.axon_hooks")
    mod.set_axon_ntff_profile_hook = lambda h: holder.__setitem__("hook", h)
    mod.get_axon_ntff_profile_hook = lambda: holder["hook"]
    sys.modules["antenv.axon_hooks"] = mod
    try:
        import antenv

        antenv.axon_hooks = mod
    except ImportError:
        pass
    holder["hook"] = _make("/opt/axon/libaxon_pjrt.so")


def _split_multiwait(nc, max_waits=1):
    """This walrus build rejects >1 sync wait per instruction; split extras
    onto same-engine NoOps inserted just before (per-engine order is the
    execution order, so semantics are preserved)."""
    from concourse import mybir

    k = [0]
    for fn in nc.m.functions:
        for blk in fn.blocks:
            out = []
            for inst in blk.instructions:
                si = getattr(inst, "sync_info", None)
                if si is not None and len(si.on_wait) > max_waits:
                    waits = list(si.on_wait)
                    for w in waits[max_waits:]:
                        k[0] += 1
                        out.append(
                            mybir.InstNoOp(
                                name=f"{inst.name}-mw{k[0]}",
                                sync_info=mybir.SyncInfo(on_wait=[w], on_update=[]),
                                bass_nofuse=True,
                                engine=inst.engine,
                            )
                        )
                    inst.sync_info = mybir.SyncInfo(
                        on_wait=waits[:max_waits], on_update=list(si.on_update)
                    )
                out.append(inst)
            blk.instructions[:] = out


# ---------------------------------------------------------------------------
# program builder
# ---------------------------------------------------------------------------
def build_program():
    import concourse.bass as bass
    import concourse.tile as tile
    from concourse import mybir

    # first group = just the halo chunk so the pipeline starts fast
    stat_slices = [(0, L)]
    o = L
    while o < NW:
        w = min(GSZ * L, NW - o)
        stat_slices.append((o, w))
        o += w
    f32 = mybir.dt.float32
    bf16 = mybir.dt.bfloat16
    out_dt = bf16 if OUT_BF16 else f32
    Op = mybir.AluOpType
    Act = mybir.ActivationFunctionType

    nc = bass.Bass(
        "TRN2",
        target_bir_lowering=False,
        debug=False,
        enable_asserts=False,
        num_devices=N_CORES,
    )
    xs_d = nc.dram_tensor("xs", [C, NW], f32, kind="ExternalInput").ap()
    cp_d = nc.dram_tensor("cpack", [128, 3460], bf16, kind="ExternalInput").ap()
    on_d = nc.dram_tensor("onesf", [128, 128], f32, kind="ExternalInput").ap()
    out_d = nc.dram_tensor("out_t", [C, NHALF], out_dt, kind="ExternalOutput").ap()

    with tile.TileContext(nc) as tc:
        with contextlib.ExitStack() as ctx:
            pers = ctx.enter_context(tc.tile_pool(name="pers", bufs=1))
            sq_pool = ctx.enter_context(tc.tile_pool(name="sqp", bufs=2))
            ps_pool = ctx.enter_context(tc.tile_pool(name="ps", bufs=1, space="PSUM"))
            st_pool = ctx.enter_context(tc.tile_pool(name="stats", bufs=2))
            xh_pool = ctx.enter_context(tc.tile_pool(name="xhp", bufs=3))
            s_pool = ctx.enter_context(tc.tile_pool(name="sp", bufs=3))
            out_pool = ctx.enter_context(tc.tile_pool(name="outp", bufs=3))
            rc_pool = ctx.enter_context(tc.tile_pool(name="rcp", bufs=3))

            # one packed const DMA (each dma_start costs ~1us of queue
            # time, so batch everything); f32 regions ride as bf16 bitcasts
            f32r = mybir.dt.float32r
            cpack = pers.tile([128, 3460], bf16, tag="cpack")
            xb = pers.tile([128, CT * NW], f32r, tag="xb")
            zc = pers.tile([128, CT * NW], bf16, tag="zc")
            xs3 = xs_d.bitcast(f32r).rearrange("(ct p) n -> p ct n", ct=CT)
            xb3 = xb[:].rearrange("p (ct n) -> p ct n", ct=CT)  # f32r view
            xbf3 = xb[:].bitcast(f32).rearrange("p (ct n) -> p ct n", ct=CT)
            zc3 = zc[:].rearrange("p (ct n) -> p ct n", ct=CT)

            def load_group(g, queue):
                o, wd = stat_slices[g]
                queue.dma_start(
                    out=xb3[:, :, o : o + wd], in_=xs3[:, :, o : o + wd]
                )

            # sync ring: g0, ones (tiny, unblocks the first mean-matmul),
            # g2, g4; scalar ring: g1, cpack, g3
            load_group(0, nc.sync)
            ones_t = pers.tile([128, 128], f32r, tag="ones")
            nc.sync.dma_start(out=ones_t[:], in_=on_d.bitcast(f32r))
            load_group(1, nc.scalar)
            nc.scalar.dma_start(out=cpack[:], in_=cp_d)
            load_group(2, nc.sync)
            load_group(3, nc.scalar)
            load_group(4, nc.sync)
            T5 = [cpack[:, b * 128 : (b + 1) * 128] for b in range(R)]
            P2 = [cpack[:, 512 + b * 128 : 512 + (b + 1) * 128] for b in range(R)]
            W5 = [cpack[:, 1024 + i * 512 : 1024 + (i + 1) * 512] for i in range(CT)]
            ident = cpack[:, 3072:3200]
            onecol = cpack[:, 3200:3202]
            ones = ones_t[:]
            epsb = cpack[:, 3458:3460].bitcast(f32)
            actwarm = pers.tile([128, 1], f32, tag="actwarm")
            nc.scalar.activation(
                out=actwarm[:], in_=ones_t[:, 0:1].bitcast(f32), func=Act.Exp
            )

            rcols = {}

            def emit_stats(g):
                o, wd = stat_slices[g]
                nk = wd // L
                ps_m = ps_pool.tile([128, 512], f32, tag="misc", bufs=2)
                for ct in range(CT):
                    nc.tensor.matmul(
                        out=ps_m[:, :wd], lhsT=ones,
                        rhs=xb3[:, ct, o : o + wd],
                        start=(ct == 0), stop=(ct == CT - 1),
                    )
                m_rep = st_pool.tile([128, 512], f32, tag="meanbf")
                nc.scalar.activation(out=m_rep[:, :wd], in_=ps_m[:, :wd], func=Act.Copy)
                zsq = sq_pool.tile([128, CT * 512], bf16, tag="xsq", name=f"zsq{g}")
                zsq3 = zsq[:].rearrange("p (ct n) -> p ct n", ct=CT)
                for ct in range(CT):
                    zeng = nc.vector if (g == 0 or ct < 2) else nc.gpsimd
                    zeng.tensor_tensor(
                        out=zc3[:, ct, o : o + wd], in0=xbf3[:, ct, o : o + wd],
                        in1=m_rep[:, :wd], op=Op.subtract,
                    )
                    eng = nc.vector if (g == 0 or ct % 2 == 0) else nc.gpsimd
                    eng.tensor_tensor(
                        out=zsq3[:, ct, :wd], in0=zc3[:, ct, o : o + wd],
                        in1=zc3[:, ct, o : o + wd], op=Op.mult,
                    )
                scol_ps = ps_pool.tile([128, nk], f32, tag="misc", bufs=2)
                for kk in range(nk):
                    for ct in range(CT):
                        nc.tensor.matmul(
                            out=scol_ps[:, kk : kk + 1],
                            lhsT=zsq3[:, ct, kk * L : (kk + 1) * L],
                            rhs=onecol[:, 1:2],
                            start=(ct == 0), stop=(ct == CT - 1),
                        )
                var = st_pool.tile([128, nk], f32, tag="varc")
                nc.vector.tensor_scalar(
                    out=var[:], in0=scol_ps[:], scalar1=1.0, scalar2=None,
                    op0=Op.mult,
                )
                lnv = st_pool.tile([128, nk], f32, tag="lnvc")
                nc.scalar.activation(out=lnv[:], in_=var[:], func=Act.Ln, bias=epsb)
                rc = rc_pool.tile([128, nk], f32, tag="rcol", name=f"rcol{g}")
                nc.scalar.activation(out=rc[:], in_=lnv[:], func=Act.Exp, scale=-0.5)
                rcols[g] = rc

            def grp(k):
                return 0 if k == 0 else (k - 1) // GSZ + 1

            def r_col(k):
                g = grp(k)
                kk = 0 if k == 0 else (k - 1) % GSZ
                return rcols[g][:, kk : kk + 1]

            def zc_slice(k, dt):
                return zc3[:, dt, k * L : (k + 1) * L]

            def make_xh(k):
                """scaled transposes: xh cols = dt*512 + b*128 + c.
                One LDW per dtile."""
                xh = xh_pool.tile([128, 4 * 512], bf16, tag="xh")
                for dp in range(2):
                    sp = ps_pool.tile([128, 1024], f32, tag="xps", bufs=2,
                                      name=f"xps{k}_{dp}")
                    for dd in range(2):
                        dt = dp * 2 + dd
                        nc.tensor.matmul(
                            out=sp[:, dd * 512 : (dd + 1) * 512],
                            lhsT=zc_slice(k, dt), rhs=W5[dt],
                            start=True, stop=True,
                        )
                    dst = xh[:, dp * 1024 : (dp + 1) * 1024]
                    if dp == 0:
                        nc.vector.tensor_scalar(
                            out=dst, in0=sp[:], scalar1=r_col(k), scalar2=None,
                            op0=Op.mult,
                        )
                    else:
                        nc.scalar.activation(
                            out=dst, in_=sp[:], func=Act.Copy, scale=r_col(k)
                        )
                return xh[:].rearrange("p (dt b c) -> p dt b c", dt=CT, b=4)

            def rhs_b(xh4, b):
                return xh4[:, :, b, :]

            def chunk_tail(k, ema_ps, ot, half):
                s_sb = s_pool.tile([128, 512], bf16, tag="ssb")
                nc.scalar.activation(out=s_sb[:], in_=ema_ps[:], func=Act.Copy)
                t_ps = ps_pool.tile([128, 512], f32, tag="ema", bufs=2)
                for dt in range(CT):
                    nc.tensor.matmul(
                        out=t_ps[:, dt * 128 : (dt + 1) * 128],
                        lhsT=s_sb[:, dt * 128 : (dt + 1) * 128], rhs=ident,
                        start=True, stop=True,
                    )
                resid = xbf3[:, :, k * L : (k + 1) * L]
                ot3 = ot[:].rearrange("p (dt i) -> p dt i", dt=CT)
                nc.vector.tensor_tensor(
                    out=ot3[:, :, half * L : (half + 1) * L],
                    in0=t_ps[:].rearrange("p (dt i) -> p dt i", dt=CT),
                    in1=resid, op=Op.add,
                )

            # ---- emission: stats groups interleaved with chunk pairs ----
            ks = list(range(K0, NCH))
            pairs = [ks[i : i + 2] for i in range(0, len(ks), 2)]
            emitted = set()

            def need_group(g):
                if g not in emitted and g < len(stat_slices):
                    emitted.add(g)
                    emit_stats(g)

            need_group(0)
            prev = None
            for k in range(K0):  # halo chunks: correction source only
                prev = make_xh(k)

            for pair in pairs:
                for k in pair:
                    need_group(grp(k))
                xhs, psums = [], []
                for k in pair:
                    xhs.append(make_xh(k))
                prevs = [prev, xhs[0]]
                for b in range(R):  # this-chunk triangular, pair-interleaved
                    for i, k in enumerate(pair):
                        if b == 0:
                            psums.append(ps_pool.tile([128, 512], f32, tag="ema",
                                                      bufs=2, name=f"emaps{k}"))
                        nc.tensor.matmul(
                            out=psums[i][:], lhsT=T5[b], rhs=rhs_b(xhs[i], b),
                            start=(b == 0), stop=False,
                        )
                for b in range(R):  # previous-chunk correction
                    for i, k in enumerate(pair):
                        nc.tensor.matmul(
                            out=psums[i][:], lhsT=P2[b], rhs=rhs_b(prevs[i], b),
                            start=False, stop=(b == R - 1),
                        )
                prev = xhs[-1]
                ot = out_pool.tile([128, CT * 2 * L], out_dt, tag="out")
                for i, k in enumerate(pair):
                    chunk_tail(k, psums[i], ot, i)
                ko = pair[0] - K0
                nc.sync.dma_start(
                    out=out_d.rearrange("(dt p) n -> p dt n", dt=CT)[
                        :, :, ko * L : (ko + 2) * L
                    ],
                    in_=ot[:].rearrange("p (dt i) -> p dt i", dt=CT),
                )
                need_group(grp(pair[1] + 2))  # prefetch: emitted after PE work
    return nc


def _host_params(ln_gamma, ln_beta, expansion, reduction, alphas, dampen_factors):
    import ml_dtypes

    a = 1.0 / (1.0 + np.exp(-alphas.astype(np.float64)))
    q = (1.0 - a) / (1.0 + np.exp(-dampen_factors.astype(np.float64)))
    qmax = float(q.max())
    assert qmax**W < 1e-8, f"halo W={W} too small for qmax={qmax}"
    rho = (  # WITHOUT a_h: amplitude lives in the kernel matrix M
        expansion.astype(np.float64)
        * reduction.astype(np.float64)
        * ln_gamma.astype(np.float64)[None, :]
    )  # [H, C]
    lag = np.arange(LAGS)
    M = a[:, None] * (q[:, None] ** lag[None, :])  # [H, LAGS]
    U, S, Vt = np.linalg.svd(M, full_matrices=False)
    beta = U[:, :R] * S[:R]  # [H, R]
    phi = Vt[:R]  # [R, LAGS]
    cb = np.einsum("hd,hb->bd", rho, beta)  # [R, C]

    bf = ml_dtypes.bfloat16
    ii, jj = np.meshgrid(np.arange(L), np.arange(L), indexing="ij")
    t5 = np.zeros((R * 128, 128), bf)
    p2 = np.zeros((R * 128, 128), bf)
    for b in range(R):
        lagm = ii - jj
        Tb = np.where(lagm >= 0, phi[b][np.clip(lagm, 0, LAGS - 1)], 0.0)
        t5[b * 128 : (b + 1) * 128, :] = Tb.T.astype(bf)  # lhsT[j,i]
        lag2 = ii + L - jj  # lag from previous chunk, in [1, 255]
        P2b = np.where(lag2 < LAGS, phi[b][np.clip(lag2, 0, LAGS - 1)], 0.0)
        p2[b * 128 : (b + 1) * 128, :] = P2b.T.astype(bf)
    w5 = np.zeros((CT * 128, 512), bf)
    for dt in range(CT):
        blk = np.zeros((128, 512))
        for b in range(R):
            blk[:, b * 128 : (b + 1) * 128] = np.diag(cb[b, dt * 128 : (dt + 1) * 128])
        w5[dt * 128 : (dt + 1) * 128, :] = blk.astype(bf)
    ident = np.eye(128, dtype=bf)
    onecol = np.zeros((128, 2), bf)
    onecol[:, 0] = 1.0 / 128.0
    onecol[:, 1] = 1.0 / C
    onesf = np.full((128, 128), 1.0 / C, np.float32)
    epsf = np.full((128, 1), EPS, np.float32)
    cpack = np.zeros((128, 3460), bf)
    for b in range(R):
        cpack[:, b * 128 : (b + 1) * 128] = t5[b * 128 : (b + 1) * 128, :]
        cpack[:, 512 + b * 128 : 512 + (b + 1) * 128] = p2[b * 128 : (b + 1) * 128, :]
    for dt in range(CT):
        cpack[:, 1024 + dt * 512 : 1024 + (dt + 1) * 512] = w5[dt * 128 : (dt + 1) * 128, :]
    cpack[:, 3072:3200] = ident
    cpack[:, 3200:3202] = onecol
    cpack[:, 3202:3458] = onesf.view(bf)
    cpack[:, 3458:3460] = epsf.view(bf)
    consts = dict(cpack=cpack, onesf=onesf)
    return a, q, consts


def _beta_term(ln_beta, expansion, reduction, a, q):
    if not np.any(ln_beta):
        return None
    n_idx = np.arange(N, dtype=np.float64)
    Cn = a[:, None] * (1.0 - q[:, None] ** (n_idx[None, :] + 1.0)) / (1.0 - q[:, None])
    w = (
        expansion.astype(np.float64)
        * reduction.astype(np.float64)
        * ln_beta.astype(np.float64)[None, :]
    )
    return np.einsum("hc,hn->cn", w, Cn).astype(np.float32)


def _make_in_maps(x, consts):
    in_maps = []
    for core in range(N_CORES):
        b, half = divmod(core, 2)
        xs = np.zeros((C, NW), np.float32)
        s = half * NHALF - W
        if s < 0:
            xs[:, W:] = x[b, :, :NHALF]
        else:
            xs[:] = x[b, :, s : s + NW]
        in_maps.append(dict(consts, xs=xs))
    return in_maps


def kernel(x, ln_gamma, ln_beta, expansion, reduction, alphas, dampen_factors,
           trace=False):
    _install_ntff_shim()
    from concourse.bass_utils import run_bass_kernel_spmd
    from concourse.bass_interp import get_hw_module

    x = np.asarray(x, np.float32)
    a, q, consts = _host_params(
        np.asarray(ln_gamma), np.asarray(ln_beta), np.asarray(expansion),
        np.asarray(reduction), np.asarray(alphas), np.asarray(dampen_factors),
    )
    nc = build_program()
    _split_multiwait(nc)
    nc.m = get_hw_module(nc.m)

    in_maps = _make_in_maps(x, consts)
    res = run_bass_kernel_spmd(
        nc, in_maps, core_ids=list(range(N_CORES)), trace=trace
    )

    out = np.empty((B, C, N), np.float32)
    for core in range(N_CORES):
        b, half = divmod(core, 2)
        out[b, :, half * NHALF : (half + 1) * NHALF] = np.asarray(
            res.results[core]["out_t"], np.float32
        )
    bt = _beta_term(
        np.asarray(ln_beta), np.asarray(expansion), np.asarray(reduction), a, q
    )
    if bt is not None:
        out += bt[None]
    if trace:
        kernel.last_results = res
    return out
